# revision 2
# baseline (speedup 1.0000x reference)
"""Multi-head attention (B=2, L=2048, D=1024, H=16) on 8 trn2 NeuronCores — v2.

Sharding: core c = (batch b = c // 4, head-group g = c % 4); each group owns 4
heads (256 dims). Q/K/V projections column-parallel per group, attention fully
local per (batch, head), fc row-parallel with the 4 group partials summed on
host. HW timeline is software-pipelined so ScalarE (softmax exp — the
bottleneck engine) starts ~20us in and never starves:

  head:  kproj.h0 -> qproj s0,s1 -> vproj lbs0-3  (PE dense; ScalarE does
         q/k evacuations, DVE does v evacuations in the shadow)
  attn:  per span s (hi-major over head pairs), per kb unit:
         2 row-tiled score MMs (head pair concurrent in the PE array)
         -> one exp [128, 2, 512-c0] on ScalarE (or DVE int16 fast-exp for
            trimmed units) -> diag-mask muls on GpSimd -> 2 PV MMs.
         Remaining projections and fc interleave into the PE stream.
  norm:  denom row -> reciprocal_approx_fast (DVE, reads PSUM directly) ->
         bf16 -> GpSimd partition_broadcast -> 2 DVE muls into ctx.
  fc:    per lb: 4 MMs -> one [128,1024] DVE evac (bf16) -> DMA out.
"""

import numpy as np
import ml_dtypes

import concourse.bass as bass
import concourse.mybir as mybir
import concourse.tile as tile
from concourse import bacc, bass_utils, library_config

L = 2048
D = 1024
DK = 64
GH = 4            # heads per core
DG = 256          # dims per core
NB = L // 128     # 16 key/query blocks
NSPAN = L // 512  # 4 query spans
F32 = mybir.dt.float32
BF = mybir.dt.bfloat16
I16 = mybir.dt.int16
U8 = mybir.dt.uint8

LOG2E = 1.4426950408889634
# units with exp free-dim <= this go to DVE via the int16 fast-exp trick
DVE_EXP_FDMAX = 768
# additive bias for the fast-exp int16 conversion (0.5 if HW truncates)
FEXP_BIAS = 16256.0

_CACHE: dict = {}
LAST_EXEC_NS = None
TRACE = False


def _install_ntff_hook():
    """Register the axon NTFF profiling hook that this image's antenv lacks."""
    import contextlib
    import ctypes
    import sys
    import types

    try:
        from antenv.axon_hooks import get_axon_ntff_profile_hook  # noqa: F401
        return
    except ImportError:
        pass
    import antenv

    mod = types.ModuleType("antenv.axon_hooks")
    state = {"hook": None}
    mod.set_axon_ntff_profile_hook = lambda h: state.__setitem__("hook", h)
    mod.get_axon_ntff_profile_hook = lambda: state["hook"]
    sys.modules["antenv.axon_hooks"] = mod
    antenv.axon_hooks = mod

    so_path = "/opt/axon/libaxon_pjrt.so"
    lib = ctypes.CDLL(so_path)
    if not hasattr(lib, "axon_start_nrt_profile"):
        return
    lib.axon_start_nrt_profile.argtypes = [
        ctypes.POINTER(ctypes.c_int64),
        ctypes.c_size_t,
    ]
    lib.axon_start_nrt_profile.restype = ctypes.c_int64
    lib.axon_stop_nrt_profile.argtypes = [ctypes.c_char_p]
    lib.axon_stop_nrt_profile.restype = ctypes.c_int64

    @contextlib.contextmanager
    def _hook(output_dir, device_ids):
        import jax

        jax.devices()
        if device_ids:
            ids = (ctypes.c_int64 * len(device_ids))(*device_ids)
            rc = lib.axon_start_nrt_profile(ids, len(device_ids))
        else:
            rc = lib.axon_start_nrt_profile(None, 0)
        if rc != 0:
            raise RuntimeError(f"axon_start_nrt_profile rc={rc}")
        try:
            yield
        finally:
            n = lib.axon_stop_nrt_profile(str(output_dir).encode())
            print(f"profile: {n} file(s) written to {output_dir}", file=sys.stderr)

    state["hook"] = _hook


def _classify(mask2d: np.ndarray) -> np.ndarray:
    """cls[qb, kb]: 0 = all masked (dead), 1 = all unmasked (pure), 2 = mixed."""
    m = mask2d.astype(np.uint8).reshape(NB, 128, NB, 128)
    s = m.sum(axis=(1, 3))
    cls = np.full((NB, NB), 2, np.int8)
    cls[s == 0] = 0
    cls[s == 128 * 128] = 1
    return cls


def _mixed_list(cls):
    return [(qb, kb) for qb in range(NB) for kb in range(NB) if cls[qb, kb] == 2]


def _build(cls: np.ndarray, zq: bool, zk: bool, zv: bool, zf: bool):
    nc = bacc.Bacc("TRN2", target_bir_lowering=False, debug=False, num_devices=8)
    XTQ = nc.dram_tensor("XTQ", [D, L], BF, kind="ExternalInput").ap()
    XTK = nc.dram_tensor("XTK", [D, L], BF, kind="ExternalInput").ap()
    XTV = nc.dram_tensor("XTV", [D + 1, L], BF, kind="ExternalInput").ap()
    WQT = nc.dram_tensor("WQT", [D, DG], BF, kind="ExternalInput").ap()
    WKT = nc.dram_tensor("WKT", [D, DG], BF, kind="ExternalInput").ap()
    WVT = nc.dram_tensor("WVT", [D + 1, DG], BF, kind="ExternalInput").ap()
    BQ = nc.dram_tensor("BQ", [DG, 1], F32, kind="ExternalInput").ap()
    BK = nc.dram_tensor("BK", [DG, 1], F32, kind="ExternalInput").ap()
    FCT = nc.dram_tensor("FCT", [DG + 1, D], BF, kind="ExternalInput").ap()
    mixed = _mixed_list(cls)
    nmix = max(1, len(mixed))
    MCHUNKS = nc.dram_tensor("MCHUNKS", [nmix, 128, 128], U8, kind="ExternalInput").ap()
    Y = nc.dram_tensor("Y", [L, D], BF, kind="ExternalOutput").ap()

    # per-span live key blocks (shared by all heads; mask broadcasts over heads)
    span_kbs = []
    span_c0 = []
    for s in range(NSPAN):
        kbs, c0s = [], {}
        for kb in range(NB):
            live = [j for j in range(4) if cls[4 * s + j, kb]]
            if live:
                kbs.append(kb)
                c0s[kb] = live[0] * 128
        assert kbs, f"query span {s} has no unmasked keys"
        span_kbs.append(kbs)
        span_c0.append(c0s)
    m01_idx = {qk: i for i, qk in enumerate(mixed)}

    Exp = mybir.ActivationFunctionType.Exp

    with tile.TileContext(nc) as tc:
        with (
            tc.tile_pool(name="w", bufs=1) as wp,
            tc.tile_pool(name="ptp", bufs=4) as ptp,
            tc.tile_pool(name="sm", bufs=2) as smp,
            tc.tile_pool(name="ys", bufs=2) as ysp,
            tc.tile_pool(name="pa", bufs=2, space="PSUM") as pa,
            tc.tile_pool(name="po", bufs=2, space="PSUM") as po,
        ):
            nc.gpsimd.load_library(library_config.proxy)

            # ---------------- persistent tiles ----------------
            qt_sb = [wp.tile([128, L], BF, tag=f"qt{i}", name=f"qt{i}") for i in range(2)]
            kt_sb = [wp.tile([128, L], BF, tag=f"kt{i}", name=f"kt{i}") for i in range(2)]
            ctx_sb = [wp.tile([128, L], BF, tag=f"ctx{i}", name=f"ctx{i}") for i in range(2)]
            v_sb = wp.tile([128, NB, GH, 128], BF, tag="vsb")
            xk = wp.tile([128, 8, L], BF, tag="xk")
            xv = wp.tile([128, 8, L], BF, tag="xv")
            xq = wp.tile([128, 8, L], BF, tag="xq")

            zeros = wp.tile([128, 64], F32, tag="zeros")
            nc.vector.memset(zeros[:], 0.0)
            nc.vector.memset(v_sb[:, :, :, 64:128], 0.0)
            nc.scalar.add(
                v_sb[:, :, :, 64:65],
                zeros[:].rearrange("p (a b c) -> p a b c", a=NB, b=GH),
                1.0,
            )
            if not zf:
                zrow = wp.tile([1, L], F32, tag="zrow")
                nc.vector.memset(zrow[:], 0.0)
                ctx1 = wp.tile([1, L], BF, tag="ctx1")
                nc.scalar.add(ctx1[:], zrow[:], 1.0)

            # ---------------- weights + x DMAs (priority order) -------------
            wqt = wp.tile([128, 8, DG], BF, tag="wqt")
            wkt = wp.tile([128, 8, DG], BF, tag="wkt")
            wvt = wp.tile([128, 8, DG], BF, tag="wvt")
            fct = wp.tile([128, 2, D], BF, tag="fct")

            def dma_x(dst, src, half):
                for kt in range(8):
                    nc.sync.dma_start(
                        out=dst[:, kt, half * 1024:(half + 1) * 1024],
                        in_=src[kt * 128:(kt + 1) * 128,
                                half * 1024:(half + 1) * 1024],
                    )

            nc.sync.dma_start(out=wkt[:], in_=WKT.rearrange("(kt p) m -> p kt m", p=128))
            dma_x(xk, XTK, 0)
            nc.sync.dma_start(out=wqt[:], in_=WQT.rearrange("(kt p) m -> p kt m", p=128))
            dma_x(xq, XTQ, 0)
            nc.sync.dma_start(out=wvt[:], in_=WVT[0:D].rearrange("(kt p) m -> p kt m", p=128))
            dma_x(xv, XTV, 0)
            dma_x(xk, XTK, 1)
            dma_x(xv, XTV, 1)
            dma_x(xq, XTQ, 1)
            nc.sync.dma_start(out=fct[:], in_=FCT[0:DG].rearrange("(i p) m -> p i m", p=128))

            if not zq:
                bq = wp.tile([128, 2, 1], F32, tag="bq")
                nc.sync.dma_start(out=bq[:], in_=BQ.rearrange("(i p) o -> p i o", p=128))
            if not zk:
                bk = wp.tile([128, 2, 1], F32, tag="bk")
                nc.sync.dma_start(out=bk[:], in_=BK.rearrange("(i p) o -> p i o", p=128))
            if not zv:
                vrow = wp.tile([1, DG], BF, tag="vrow")
                xr = wp.tile([1, L], BF, tag="xr")
                nc.sync.dma_start(out=vrow[:], in_=WVT[D:D + 1])
                nc.sync.dma_start(out=xr[:], in_=XTV[D:D + 1])
            if not zf:
                fcb = wp.tile([1, D], BF, tag="fcb")
                nc.sync.dma_start(out=fcb[:], in_=FCT[DG:DG + 1])

            # 0/1 mask chunks for mixed blocks
            m01_all = wp.tile([128, nmix, 128], BF, tag="m01")
            if mixed:
                mstage = wp.tile([128, nmix, 128], U8, tag="mstage")
                nc.sync.dma_start(out=mstage[:], in_=MCHUNKS.rearrange("n p c -> p n c"))
                nc.scalar.copy(m01_all[:], mstage[:])

            # ---------------- projection helpers ----------------
            def proj_qk(xt, wt, bias, dst, s):
                """Project query-span (or key-span) s: out dims 256 x 512 cols."""
                for mch in range(2):
                    p = pa.tile([128, 1024], F32, tag="ps", name=f"pqk{s}_{mch}")
                    for kt in range(8):
                        nc.tensor.matmul(
                            p[:, 0:512],
                            wt[:, kt, mch * 128:(mch + 1) * 128],
                            xt[:, kt, s * 512:(s + 1) * 512],
                            start=(kt == 0),
                            stop=(kt == 7),
                        )
                    if bias is None:
                        nc.scalar.copy(dst[mch][:, s * 512:(s + 1) * 512], p[:, 0:512])
                    else:
                        nc.vector.tensor_scalar_add(
                            dst[mch][:, s * 512:(s + 1) * 512], p[:, 0:512],
                            bias[:, mch],
                        )

            def proj_v(lb):
                p = pa.tile([128, 1024], F32, tag="ps", name=f"pv{lb}")
                for kt in range(8):
                    nc.tensor.matmul(
                        p[:, 0:DG],
                        xv[:, kt, lb * 128:(lb + 1) * 128],
                        wvt[:, kt],
                        start=(kt == 0),
                        stop=(zv and kt == 7),
                    )
                if not zv:
                    nc.tensor.matmul(
                        p[:, 0:DG], xr[:, lb * 128:(lb + 1) * 128], vrow[:],
                        start=False, stop=True,
                    )
                nc.scalar.copy(
                    v_sb[:, lb, :, 0:64], p[:, 0:DG].rearrange("p (h d) -> p h d", h=GH)
                )

            def fc_lb(lb):
                yp = pa.tile([128, 1024], F32, tag="ps", name=f"yp{lb}")
                for nh in range(2):
                    for i in range(2):
                        nc.tensor.matmul(
                            yp[:, nh * 512:(nh + 1) * 512],
                            ctx_sb[i][:, lb * 128:(lb + 1) * 128],
                            fct[:, i, nh * 512:(nh + 1) * 512],
                            start=(i == 0), stop=(zf and i == 1),
                            skip_group_check=True,
                        )
                    if not zf:
                        nc.tensor.matmul(
                            yp[:, nh * 512:(nh + 1) * 512],
                            ctx1[:, lb * 128:(lb + 1) * 128],
                            fcb[:, nh * 512:(nh + 1) * 512],
                            start=False, stop=True,
                            skip_group_check=True,
                        )
                ys = ysp.tile([128, 1024], BF, tag="ys", name=f"ys{lb}")
                nc.vector.tensor_copy(ys[:], yp[:])
                nc.sync.dma_start(out=Y[lb * 128:(lb + 1) * 128, :], in_=ys[:])

            # ---------------- attention ----------------
            outp_tiles = {}

            def scores_block(s, hi, kb):
                c0 = span_c0[s][kb]
                scp = pa.tile([128, 1024], F32, tag="ps", name=f"sc{s}_{hi}_{kb}")
                for h01 in range(2):
                    ho = h01 * 64
                    nc.tensor.matmul(
                        scp[:, h01 * 512 + c0:(h01 + 1) * 512],
                        kt_sb[hi][ho:ho + 64, kb * 128:(kb + 1) * 128],
                        qt_sb[hi][ho:ho + 64, s * 512 + c0:(s + 1) * 512],
                        start=True, stop=True,
                    )
                return scp

            def exp_block(s, hi, kb, scp):
                c0 = span_c0[s][kb]
                fd = 2 * (512 - c0)
                pt = ptp.tile([128, 1024], BF, tag="pt", name=f"pt{s}_{hi}_{kb}")
                sc3 = scp[:].rearrange("p (h q) -> p h q", h=2)
                pt3 = pt[:].rearrange("p (h q) -> p h q", h=2)
                if fd <= DVE_EXP_FDMAX:
                    pti = pt[:].bitcast(I16).rearrange("p (h q) -> p h q", h=2)
                    nc.vector.tensor_scalar(
                        out=pti[:, :, c0:512], in0=sc3[:, :, c0:512],
                        scalar1=16.0 * LOG2E, scalar2=FEXP_BIAS,
                        op0=mybir.AluOpType.mult, op1=mybir.AluOpType.add,
                    )
                else:
                    nc.scalar.activation(
                        pt3[:, :, c0:512], sc3[:, :, c0:512], Exp, scale=0.125
                    )
                for j in range(c0 // 128, 4):
                    qb = 4 * s + j
                    c = cls[qb, kb]
                    if c == 0:
                        nc.vector.memset(pt3[:, :, j * 128:(j + 1) * 128], 0.0)
                    elif c == 2:
                        for h01 in range(2):
                            nc.gpsimd.tensor_tensor(
                                out=pt3[:, h01, j * 128:(j + 1) * 128],
                                in0=pt3[:, h01, j * 128:(j + 1) * 128],
                                in1=m01_all[:, m01_idx[(qb, kb)], :],
                                op=mybir.AluOpType.mult,
                            )
                return pt

            fillers = []

            def attn_pair(s, hi, kbpair):
                """Double unit: scores+exp for 2 kbs, one filler chain while
                ScalarE computes the exps, then per-head adjacent PV chain
                segments so LDWEIGHTS hides under the previous MM."""
                scps = [(kb, scores_block(s, hi, kb)) for kb in kbpair]
                kb_pts = [(kb, exp_block(s, hi, kb, scp)) for kb, scp in scps]
                if fillers:
                    fillers.pop(0)()
                outp = outp_tiles[(s, hi)]
                kbs = span_kbs[s]
                for h01 in range(2):
                    for kb, pt in kb_pts:
                        c0 = span_c0[s][kb]
                        nc.tensor.matmul(
                            outp[:, h01 * 512 + c0:(h01 + 1) * 512],
                            v_sb[:, kb, 2 * hi + h01, :],
                            pt[:, h01 * 512 + c0:(h01 + 1) * 512],
                            start=(kb == kbs[0]), stop=(kb == kbs[-1]),
                            skip_group_check=True,
                        )

            def attn_span_units(s, hi):
                outp_tiles[(s, hi)] = po.tile(
                    [128, 1024], F32, tag="po", name=f"u{s}_{hi}"
                )
                kbs = span_kbs[s]
                return [tuple(kbs[i:i + 2]) for i in range(0, len(kbs), 2)]

            def normalize(s, hi):
                outp = outp_tiles.pop((s, hi))
                drow = smp.tile([1, 1024], F32, tag="drow", name=f"dr{s}_{hi}")
                nc.vector.tensor_copy(drow[:], outp[64:65, :])
                rrow = smp.tile([1, 1024], F32, tag="rrow", name=f"rr{s}_{hi}")
                nc.vector.reciprocal_approx_fast(out=rrow[:], in_=drow[:])
                rbf = smp.tile([1, 1024], BF, tag="rbf", name=f"rb{s}_{hi}")
                nc.vector.tensor_copy(rbf[:], rrow[:])
                bcs = smp.tile([64, 1024], BF, tag="bcs", name=f"bc{s}_{hi}")
                nc.gpsimd.partition_broadcast(out_ap=bcs[:], in_ap=rbf[:])
                for h01 in range(2):
                    nc.vector.tensor_tensor(
                        out=ctx_sb[hi][h01 * 64:(h01 + 1) * 64, s * 512:(s + 1) * 512],
                        in0=outp[0:64, h01 * 512:(h01 + 1) * 512],
                        in1=bcs[:, h01 * 512:(h01 + 1) * 512],
                        op=mybir.AluOpType.mult,
                    )

            # ---------------- emission schedule ----------------
            bqa = None if zq else bq
            bka = None if zk else bk

            # head phase: k half0, q span0, v lbs 0-3 — attention starts ASAP
            for s in (0, 1):
                proj_qk(xk, wkt, bka, kt_sb, s)
            proj_qk(xq, wqt, bqa, qt_sb, 0)
            for lb in range(0, 4):
                proj_v(lb)

            # remaining projections + fc as per-du fillers, dependency-ordered
            fillers.append(lambda: proj_qk(xq, wqt, bqa, qt_sb, 1))
            for lb in range(4, 8):
                fillers.append(lambda lb=lb: proj_v(lb))
            fillers.append(lambda: proj_qk(xk, wkt, bka, kt_sb, 2))
            fillers.append(lambda: proj_qk(xq, wqt, bqa, qt_sb, 2))
            fillers.append(lambda: fc_lb(0))
            fillers.append(lambda: fc_lb(1))
            for lb in range(8, 12):
                fillers.append(lambda lb=lb: proj_v(lb))
            fillers.append(lambda: proj_qk(xk, wkt, bka, kt_sb, 3))
            fillers.append(lambda: fc_lb(2))
            fillers.append(lambda: fc_lb(3))
            fillers.append(lambda: proj_qk(xq, wqt, bqa, qt_sb, 3))
            fillers.append(lambda: fc_lb(4))
            fillers.append(lambda: fc_lb(5))
            for lb in range(12, 16):
                fillers.append(lambda lb=lb: proj_v(lb))
            fillers.append(lambda: fc_lb(6))
            fillers.append(lambda: fc_lb(7))
            fillers.append(lambda: None and None)
            fillers.append(lambda: fc_lb(8))
            fillers.append(lambda: fc_lb(9))
            fillers.append(lambda: fc_lb(10))
            fillers.append(lambda: fc_lb(11))

            for s in range(NSPAN):
                for hi in range(2):
                    for du in attn_span_units(s, hi):
                        attn_pair(s, hi, du)
                    normalize(s, hi)
            for lb in range(12, 16):
                fc_lb(lb)

    nc.compile()
    return nc


def kernel(Q, K, V, mask, Wq_w, Wq_b, Wk_w, Wk_b, Wv_w, Wv_b, fc_w, fc_b):
    global LAST_EXEC_NS
    Q = np.asarray(Q, np.float32)
    K = np.asarray(K, np.float32)
    V = np.asarray(V, np.float32)
    mask2d = np.asarray(mask).reshape(L, L).astype(bool)
    Wq_w = np.asarray(Wq_w, np.float32)
    Wq_b = np.asarray(Wq_b, np.float32)
    Wk_w = np.asarray(Wk_w, np.float32)
    Wk_b = np.asarray(Wk_b, np.float32)
    Wv_w = np.asarray(Wv_w, np.float32)
    Wv_b = np.asarray(Wv_b, np.float32)
    fc_w = np.asarray(fc_w, np.float32)
    fc_b = np.asarray(fc_b, np.float32)

    cls = _classify(mask2d)
    zq = not Wq_b.any()
    zk = not Wk_b.any()
    zv = not Wv_b.any()
    zf = not fc_b.any()
    key = (cls.tobytes(), zq, zk, zv, zf)
    if key not in _CACHE:
        _CACHE[key] = [_build(cls, zq, zk, zv, zf), False]
    nc, warmed = _CACHE[key]

    bf = ml_dtypes.bfloat16
    mixed = _mixed_list(cls)
    if mixed:
        mchunks = np.stack([
            np.ascontiguousarray(mask2d[qb * 128:(qb + 1) * 128, kb * 128:(kb + 1) * 128].T)
            for qb, kb in mixed
        ]).astype(np.uint8)
    else:
        mchunks = np.zeros((1, 128, 128), np.uint8)
    ones_row = np.ones((1, L), np.float32)

    xt = {}
    for b in range(2):
        xt[("Q", b)] = np.ascontiguousarray(Q[b].T).astype(bf)
        xt[("K", b)] = np.ascontiguousarray(K[b].T).astype(bf)
        xt[("V", b)] = np.concatenate([np.ascontiguousarray(V[b].T), ones_row], 0).astype(bf)

    in_maps = []
    for c in range(8):
        b, g = c // 4, c % 4
        sl = slice(g * DG, (g + 1) * DG)
        fc_last = fc_b[None, :] if g == 0 else np.zeros((1, D), np.float32)
        in_maps.append({
            "XTQ": xt[("Q", b)],
            "XTK": xt[("K", b)],
            "XTV": xt[("V", b)],
            "WQT": np.ascontiguousarray(Wq_w[sl, :].T).astype(bf),
            "WKT": np.ascontiguousarray(Wk_w[sl, :].T).astype(bf),
            "WVT": np.concatenate(
                [np.ascontiguousarray(Wv_w[sl, :].T), Wv_b[sl][None, :]], 0
            ).astype(bf),
            "BQ": np.ascontiguousarray(Wq_b[sl].reshape(DG, 1)),
            "BK": np.ascontiguousarray(Wk_b[sl].reshape(DG, 1)),
            "FCT": np.concatenate(
                [np.ascontiguousarray(fc_w[:, sl].T), fc_last], 0
            ).astype(bf),
            "MCHUNKS": mchunks,
        })

    if TRACE:
        _install_ntff_hook()
    if not warmed:
        bass_utils.run_bass_kernel_spmd(nc, in_maps, core_ids=list(range(8)))
        _CACHE[key][1] = True
    res = bass_utils.run_bass_kernel_spmd(
        nc, in_maps, core_ids=list(range(8)),
        trace=TRACE, trace_cores=list(range(8)) if TRACE else None,
    )
    LAST_EXEC_NS = res.exec_time_ns

    out = np.zeros((2, L, D), np.float32)
    for c in range(8):
        out[c // 4] += res.results[c]["Y"].astype(np.float32)
    return out
